# revision 1
# baseline (speedup 1.0000x reference)
"""GQA kernel for trn2, 8 NeuronCores.

Sharding: core c = (b, g2) with b = c//4, g2 = c%4.  Each core handles batch b
and kv heads {2*g2, 2*g2+1} (q heads 8*g2 .. 8*g2+7).  Wq/Wk/Wv column-sharded,
Wo row-sharded; host sums the 4 partial y outputs per batch (row-shard unshard).

Device dataflow (everything in "transposed" orientation so no x/q/k transposes
are ever needed on-device):
  qT[e,s] = sum_d Wq[d,e] * xT[d,s]     (Wq stationary, xT moving, full-rate f32r)
  kT likewise; vT likewise then PE-transposed to natural v[s,e].
  RoPE applied to qT/kT tiles via partition-shift DMAs + host-baked cos/sin.
  scoresT[j,i] = sum_e kT[e,j] * qT[e,i]  (two heads packed in array rows 0-63/64-127)
  expT = exp(scoresT/8) via ACT, causal-masked by DVE multiply on diagonal tiles
  outT[e,i] (+ rowsum in row 64) = sum_j v_ones[j,e|1] * expT[j,i]
  normalize by broadcasted 1/rowsum, then y[s,d] = sum_f outT[f,s] * Wo[f,d].

Q heads within a core are permuted [0,4,1,5,2,6,3,7] so that each qT partition
tile pairs one kv0-head (rows 0-63) with one kv1-head (rows 64-127), matching
kT's natural kv0|kv1 partition layout.  Wo rows are permuted identically.
"""

import os
import numpy as np

import concourse.bass as bass
import concourse.bacc as bacc
import concourse.mybir as mybir
import concourse.tile as tile
from concourse.bass_utils import run_bass_kernel_spmd

F32 = mybir.dt.float32
F32R = mybir.dt.float32r

B, S, D = 2, 2048, 2048
H, KV, HD = 32, 8, 64
N_CORES = 8
SB = 512          # s-block width (moving free dim)
NSB = S // SB     # 4
NDT = D // 128    # 16 d-tiles
NET = 4           # q e-tiles per core (512 q-cols / 128)
NIT = S // SB     # 4 i-blocks
NJT = S // 128    # 16 j-tiles
PERM = [0, 4, 1, 5, 2, 6, 3, 7]

LAST_RESULT = None  # test.py reads exec_time_ns off this


def _r(ap):
    return ap.bitcast(F32R)


def build_nc():
    nc = bacc.Bacc("TRN2", target_bir_lowering=False, debug=False,
                   enable_asserts=True, num_devices=N_CORES)

    xT = nc.dram_tensor("xT", [D, S], F32R, kind="ExternalInput")
    wq = nc.dram_tensor("wq", [D, 512], F32R, kind="ExternalInput")
    wk = nc.dram_tensor("wk", [D, 128], F32R, kind="ExternalInput")
    wv = nc.dram_tensor("wv", [D, 128], F32R, kind="ExternalInput")
    wo = nc.dram_tensor("wo", [512, D], F32R, kind="ExternalInput")
    cos2 = nc.dram_tensor("cos2", [128, S], F32, kind="ExternalInput")
    sin2 = nc.dram_tensor("sin2", [128, S], F32, kind="ExternalInput")
    cmask = nc.dram_tensor("cmask", [128, 4, SB], F32, kind="ExternalInput")
    ident = nc.dram_tensor("ident", [128, 128], F32, kind="ExternalInput")
    y = nc.dram_tensor("y", [S, D], F32, kind="ExternalOutput")
    rscratch = nc.dram_tensor("rscratch", [NIT, 8, SB], F32)  # internal

    with tile.TileContext(nc) as tc:
        with (
            tc.tile_pool(name="persist", bufs=1) as persist,
            tc.tile_pool(name="consts", bufs=1) as consts,
        ):
            # ---- persistent SBUF tensors ----
            qT_sb = [persist.tile([128, S], F32R, name=f"qT{t}") for t in range(NET)]
            kT_sb = persist.tile([128, S], F32R, name="kT")
            v_ones0 = persist.tile([128, NJT, 65], F32R, name="v_ones0")
            v_ones1 = persist.tile([128, NJT, 65], F32R, name="v_ones1")
            outT = [persist.tile([128, S], F32R, name=f"outT{t}") for t in range(NET)]
            rowsum_all = persist.tile([8, NIT, SB], F32, name="rowsum_all")
            recip_all = persist.tile([8, NIT, SB], F32, name="recip_all")
            rowsum = [rowsum_all[:, b_, :] for b_ in range(NIT)]
            recip = [recip_all[:, b_, :] for b_ in range(NIT)]

            cos_sb = consts.tile([128, S], F32, name="cos_sb")
            sin_sb = consts.tile([128, S], F32, name="sin_sb")
            cmask_sb = consts.tile([128, 4, SB], F32, name="cmask_sb")
            ident_sb = consts.tile([128, 128], F32, name="ident_sb")
            ones_col = consts.tile([128, NJT, 1], F32, name="ones_col")
            nc.gpsimd.memset(ones_col[:], 1.0)
            nc.vector.tensor_copy(v_ones0[:, :, 64:65], ones_col[:])
            nc.vector.tensor_copy(v_ones1[:, :, 64:65], ones_col[:])

            # ================= Phase A: projections =================
            with (
                tc.tile_pool(name="wpool", bufs=1) as wpool,
                tc.tile_pool(name="xpool", bufs=3) as xpool,
                tc.tile_pool(name="apsum", bufs=1, space="PSUM") as apsum,
                tc.tile_pool(name="trpsum", bufs=2, space="PSUM") as trpsum,
                tc.tile_pool(name="atmp", bufs=3) as atmp,
            ):
                wq_sb = [wpool.tile([128, 512], F32R, name=f"wq{d}") for d in range(NDT)]
                wk_sb = [wpool.tile([128, 128], F32R, name=f"wk{d}") for d in range(NDT)]
                wv_sb = [wpool.tile([128, 128], F32R, name=f"wv{d}") for d in range(NDT)]

                for sb in range(NSB):
                    scol = slice(sb * SB, (sb + 1) * SB)
                    # six accumulating psum tiles live across the d loop
                    ps_q = [apsum.tile([128, SB], F32, name=f"psq{t}", tag=f"psq{t}")
                            for t in range(NET)]
                    ps_k = apsum.tile([128, SB], F32, name="psk", tag="psk")
                    ps_v = apsum.tile([128, SB], F32, name="psv", tag="psv")
                    for d in range(NDT):
                        if sb == 0:
                            # first-use weight loads, interleaved with the x stream
                            nc.sync.dma_start(wq_sb[d][:], wq[d * 128:(d + 1) * 128, :])
                            nc.sync.dma_start(wk_sb[d][:], wk[d * 128:(d + 1) * 128, :])
                            nc.sync.dma_start(wv_sb[d][:], wv[d * 128:(d + 1) * 128, :])
                            if d == 10:
                                nc.sync.dma_start(cos_sb[:], cos2[:])
                                nc.sync.dma_start(sin_sb[:], sin2[:])
                            if d == 14:
                                nc.sync.dma_start(ident_sb[:], ident[:])
                        if sb == 1 and d == 4:
                            nc.sync.dma_start(cmask_sb[:], cmask[:])
                        xt = xpool.tile([128, SB], F32R, name="xt", tag="xt")
                        nc.sync.dma_start(xt[:], xT[d * 128:(d + 1) * 128, scol])
                        st, sp = d == 0, d == NDT - 1
                        for t in range(NET):
                            nc.tensor.matmul(ps_q[t][:], _r(wq_sb[d][:, t * 128:(t + 1) * 128]),
                                             _r(xt[:]), start=st, stop=sp)
                        nc.tensor.matmul(ps_k[:], _r(wk_sb[d][:]), _r(xt[:]), start=st, stop=sp)
                        nc.tensor.matmul(ps_v[:], _r(wv_sb[d][:]), _r(xt[:]), start=st, stop=sp)

                    # v: copy psum -> sbuf, PE-transpose 128x128 blocks, split kv heads
                    vtmp = atmp.tile([128, SB], F32, name="vtmp", tag="vtmp")
                    nc.scalar.copy(vtmp[:], ps_v[:])
                    for u in range(SB // 128):
                        jt = sb * 4 + u
                        tr = trpsum.tile([128, 128], F32, name="tr", tag="tr")
                        nc.tensor.transpose(tr[:], vtmp[:, u * 128:(u + 1) * 128], ident_sb[:])
                        nc.vector.tensor_copy(v_ones0[:, jt, 0:64], tr[:, 0:64])
                        nc.vector.tensor_copy(v_ones1[:, jt, 0:64], tr[:, 64:128])

                    # RoPE, two passes: (1) drain all PSUM accumulators to
                    # SBUF on ACT so the banks free for the next s-block ASAP,
                    # (2) shift-DMA + mul/add chains on DVE.
                    rope_src = [(ps_k, kT_sb)] + [(ps_q[t], qT_sb[t]) for t in range(NET)]
                    qtmps = []
                    for ps, _dst in rope_src:
                        qtmp = atmp.tile([128, SB], F32, name="qtmp", tag="qtmp", bufs=4)
                        nc.scalar.copy(qtmp[:], ps[:])
                        qtmps.append(qtmp)
                    for qtmp, (_ps, dst) in zip(qtmps, rope_src):
                        rot = atmp.tile([128, SB], F32, name="rot", tag="rot")
                        for (a, b_) in ((0, 32), (32, 0), (64, 96), (96, 64)):
                            nc.gpsimd.dma_start(rot[b_:b_ + 32, :], qtmp[a:a + 32, :])
                        t1 = atmp.tile([128, SB], F32, name="t1", tag="t1")
                        nc.vector.tensor_mul(t1[:], qtmp[:], cos_sb[:, scol])
                        t2 = atmp.tile([128, SB], F32, name="t2", tag="t2")
                        nc.vector.tensor_mul(t2[:], rot[:], sin_sb[:, scol])
                        nc.vector.tensor_add(dst[:, scol], t1[:], t2[:])

            # ================= Phase B: attention =================
            # bi-outer so phase C (emitted later) can overlap later bi rounds.
            # Diagonal j-tiles only compute the valid column range [lo:512);
            # the triangular 128-strip is masked with cmask[:,0,0:128].
            with (
                tc.tile_pool(name="wopool", bufs=1) as wopool,
                tc.tile_pool(name="scpsum", bufs=2, space="PSUM") as scpsum,
                tc.tile_pool(name="pvpsum", bufs=1, space="PSUM") as pvpsum,
                tc.tile_pool(name="epool", bufs=4) as epool,
                tc.tile_pool(name="btmp", bufs=3) as btmp,
                tc.tile_pool(name="ypsum", bufs=2, space="PSUM") as ypsum,
                tc.tile_pool(name="ypool", bufs=3) as ypool,
            ):
                # prefetch Wo during attention, one f-tile per bi round
                wo_sb = [wopool.tile([128, D], F32R, name=f"wo{f}") for f in range(4)]

                tri = cmask_sb[:, 0, 0:128]
                for bi in range(NIT):
                    nc.sync.dma_start(wo_sb[bi][:], wo[bi * 128:(bi + 1) * 128, :])
                    icol = slice(bi * SB, (bi + 1) * SB)
                    for t in range(NET):
                        pvA = pvpsum.tile([65, SB], F32, name="pvA", tag="pvA")
                        pvB = pvpsum.tile([65, SB], F32, name="pvB", tag="pvB")
                        njt = 4 * bi + 4
                        for jt in range(njt):
                            jcol = slice(jt * 128, (jt + 1) * 128)
                            ro = jt - 4 * bi
                            lo = 128 * max(ro, 0)
                            iband = slice(bi * SB + lo, (bi + 1) * SB)
                            scA = scpsum.tile([128, SB], F32, name="scA", tag="scA")
                            scB = scpsum.tile([128, SB], F32, name="scB", tag="scB")
                            nc.tensor.matmul(scA[:, lo:], _r(kT_sb[0:64, jcol]),
                                             _r(qT_sb[t][0:64, iband]), start=True, stop=True)
                            nc.tensor.matmul(scB[:, lo:], _r(kT_sb[64:128, jcol]),
                                             _r(qT_sb[t][64:128, iband]), start=True, stop=True)
                            eA = epool.tile([128, SB], F32R, name="eA", tag="eA")
                            eB = epool.tile([128, SB], F32R, name="eB", tag="eB")
                            nc.scalar.activation(eA[:, lo:], scA[:, lo:],
                                                 mybir.ActivationFunctionType.Exp, scale=0.125)
                            nc.scalar.activation(eB[:, lo:], scB[:, lo:],
                                                 mybir.ActivationFunctionType.Exp, scale=0.125)
                            if ro >= 0:
                                nc.vector.tensor_mul(eA[:, lo:lo + 128], eA[:, lo:lo + 128], tri)
                                nc.vector.tensor_mul(eB[:, lo:lo + 128], eB[:, lo:lo + 128], tri)
                            st, sp = jt == 0, jt == njt - 1
                            nc.tensor.matmul(pvA[:, lo:], _r(v_ones0[:, jt, :]), _r(eA[:, lo:]),
                                             start=st, stop=sp)
                            nc.tensor.matmul(pvB[:, lo:], _r(v_ones1[:, jt, :]), _r(eB[:, lo:]),
                                             start=st, stop=sp)
                        # unnormalized outT + rowsum rows (2t, 2t+1) of this bi
                        nc.vector.tensor_copy(outT[t][0:64, icol], pvA[0:64, :])
                        pvsB = btmp.tile([65, SB], F32R, name="pvsB", tag="pvsB")
                        nc.vector.tensor_copy(pvsB[:], pvB[:])
                        nc.gpsimd.dma_start(outT[t][64:128, icol], pvsB[0:64, :])
                        rowA = btmp.tile([65, SB], F32, name="rowA", tag="rowA")
                        nc.vector.tensor_copy(rowA[64:65, :], pvA[64:65, :])
                        nc.gpsimd.dma_start(rowsum[bi][2 * t:2 * t + 1, :], rowA[64:65, :])
                        nc.gpsimd.dma_start(rowsum[bi][2 * t + 1:2 * t + 2, :],
                                            pvsB[64:65, :].bitcast(F32))

                    # per-bi normalization (unblocks phase C rows 4bi..4bi+4)
                    nc.vector.reciprocal(recip[bi][:], rowsum[bi][:])
                    nc.gpsimd.dma_start(rscratch[bi], recip[bi][:])
                    for t in range(NET):
                        bc = btmp.tile([128, SB], F32, name="bc", tag="bc")
                        nc.gpsimd.dma_start(
                            bc[0:64, :],
                            rscratch[bi, 2 * t:2 * t + 1, :].broadcast_to((64, SB)))
                        nc.gpsimd.dma_start(
                            bc[64:128, :],
                            rscratch[bi, 2 * t + 1:2 * t + 2, :].broadcast_to((64, SB)))
                        nc.vector.tensor_mul(outT[t][0:64, icol],
                                             outT[t][0:64, icol], bc[0:64, :])
                        nc.vector.tensor_mul(outT[t][64:128, icol],
                                             outT[t][64:128, icol], bc[64:128, :])

            # ================= Phase C: output projection =================
                for stt in range(S // 128):
                    srow = slice(stt * 128, (stt + 1) * 128)
                    for db in range(D // SB):
                        dcol = slice(db * SB, (db + 1) * SB)
                        yp = ypsum.tile([128, SB], F32, name="yp", tag="yp")
                        for f in range(4):
                            nc.tensor.matmul(yp[:], _r(outT[f][:, srow]),
                                             _r(wo_sb[f][:, dcol]),
                                             start=(f == 0), stop=(f == 3))
                        ys = ypool.tile([128, SB], F32, name="ys", tag="ys")
                        nc.scalar.copy(ys[:], yp[:])
                        nc.sync.dma_start(y[srow, dcol], ys[:])

    nc.compile()
    return nc


def host_inputs(x, Wq, Wk, Wv, Wo):
    """Per-core input maps (8 cores)."""
    inv = 1.0 / (10000.0 ** (np.arange(0, HD, 2, dtype=np.float64) / HD))
    freqs = np.outer(np.arange(S, dtype=np.float64), inv)          # [S, 32]
    emb = np.concatenate([freqs, freqs], axis=1)                   # [S, 64]
    cos = np.cos(emb).astype(np.float32)
    sin = np.sin(emb).astype(np.float32)
    cos2 = np.ascontiguousarray(np.tile(cos.T, (2, 1)))            # [128, S]
    sinf = np.concatenate([-sin[:, :32], sin[:, 32:]], axis=1)     # sign-folded
    sin2 = np.ascontiguousarray(np.tile(sinf.T, (2, 1)))
    j = np.arange(128)[:, None, None]
    ro = np.arange(4)[None, :, None]
    i = np.arange(SB)[None, None, :]
    cmask = (j + 128 * ro <= i).astype(np.float32)                 # [128, 4, 512]
    ident = np.eye(128, dtype=np.float32)

    Wq4 = Wq.reshape(D, H, HD)
    Wo4 = Wo.reshape(H, HD, D)
    Wk4 = Wk.reshape(D, KV, HD)
    Wv4 = Wv.reshape(D, KV, HD)

    maps = []
    for c in range(N_CORES):
        b, g2 = c // 4, c % 4
        gh = [8 * g2 + p for p in PERM]
        maps.append({
            "xT": np.ascontiguousarray(x[b].T),
            "wq": np.ascontiguousarray(Wq4[:, gh, :].reshape(D, 512)),
            "wk": np.ascontiguousarray(Wk4[:, [2 * g2, 2 * g2 + 1], :].reshape(D, 128)),
            "wv": np.ascontiguousarray(Wv4[:, [2 * g2, 2 * g2 + 1], :].reshape(D, 128)),
            "wo": np.ascontiguousarray(Wo4[gh].reshape(512, D)),
            "cos2": cos2, "sin2": sin2, "cmask": cmask, "ident": ident,
        })
    return maps


_NC_CACHE = None


def kernel(x, Wq, Wk, Wv, Wo):
    global LAST_RESULT, _NC_CACHE
    x = np.asarray(x, np.float32)
    maps = host_inputs(np.asarray(x, np.float32), np.asarray(Wq, np.float32),
                       np.asarray(Wk, np.float32), np.asarray(Wv, np.float32),
                       np.asarray(Wo, np.float32))
    if _NC_CACHE is None:
        _NC_CACHE = build_nc()
    trace = bool(os.environ.get("KERNEL_TRACE"))
    try:
        res = run_bass_kernel_spmd(_NC_CACHE, maps, list(range(N_CORES)), trace=trace)
    except (ImportError, ModuleNotFoundError):
        # NTFF profile hook unavailable in this environment — run untraced.
        res = run_bass_kernel_spmd(_NC_CACHE, maps, list(range(N_CORES)), trace=False)
    LAST_RESULT = res
    out = np.empty((B, S, D), np.float32)
    for b in range(B):
        out[b] = res.results[4 * b]["y"]
        for g2 in range(1, 4):
            out[b] += res.results[4 * b + g2]["y"]
    return out



# revision 2
# speedup vs baseline: 1.3236x; 1.3236x over previous
"""GQA kernel for trn2, 8 NeuronCores — v2 (bf16, phase-interleaved).

Sharding: core c = (b, g2), b = c//4, g2 = c%4: batch b, kv heads {2g2, 2g2+1},
q heads 8g2..8g2+7 permuted [0,4,1,5,2,6,3,7] into 4 pairs t (rows 0:64 = kv0's
q head, 64:128 = kv1's).  Host sums the 4 partial y outputs per batch.

Dataflow (all matmuls bf16 in / f32 psum out):
  A: qkT[e,s] = W.T x per 256-col half-sb, e-output-major (one psum acc ring,
     2 banks); v computed in natural [s,e] orientation (no PE transpose).
     RoPE: ACT drains acc->qk5 (packed [128,5,256] bf16), 4 gpsimd shift-DMAs
     rotate all 5 tensors at once, 3 DVE ops apply cos/sin (broadcast AP).
  B: per (bi,t): fused scores psum [128,2,512] (slot A/B in different banks),
     one fused 3D exp per j-tile, DVE causal mask on diagonal, pv accumulated
     per (bi,t) into pvA[65,512] (rowsumA in row 64) / pvB[128,512] (rowsumB
     in row 32, out rows 64:128 -- partition-aligned with outT, no DMA moves).
     Normalize: DVE reciprocal -> dram round-trip broadcast (sync+gpsimd DGE)
     -> DVE mul psum x bcS -> outT bf16.
  C: y[s,d] = outT.T wo per (stt,db), ACT drain, sync DMA out.

Emission interleaves phases so the in-order PE queue never starves on ACT
(exp) or normalization chains:
  A0 A1 | B0t* x A2-units | B1t* x A3-units | B2t* x C(stt0-3) |
  B3t* x C(stt4-11...) | C tail.
PSUM: acc 2 banks (A, closes after A3) -> yp 2 banks (C); sc 4; pvA 1; pvB 1.
"""

import os
import numpy as np
import ml_dtypes

import concourse.bass as bass
import concourse.bacc as bacc
import concourse.mybir as mybir
import concourse.tile as tile
from concourse.bass_utils import run_bass_kernel_spmd

F32 = mybir.dt.float32
BF16 = mybir.dt.bfloat16

B, S, D = 2, 2048, 2048
H, KV, HD = 32, 8, 64
N_CORES = 8
NDT = D // 128          # 16 d-tiles
NIT = 4                 # 512-wide i-blocks
PERM = [0, 4, 1, 5, 2, 6, 3, 7]

LAST_RESULT = None


def build_nc():
    nc = bacc.Bacc("TRN2", target_bir_lowering=False, debug=False,
                   enable_asserts=True, num_devices=N_CORES)

    xT = nc.dram_tensor("xT", [D, S], BF16, kind="ExternalInput")
    w6 = nc.dram_tensor("w6", [D, 768], BF16, kind="ExternalInput")
    wo = nc.dram_tensor("wo", [512, D], BF16, kind="ExternalInput")
    cos2 = nc.dram_tensor("cos2", [128, S], BF16, kind="ExternalInput")
    sin2 = nc.dram_tensor("sin2", [128, S], BF16, kind="ExternalInput")
    tri2 = nc.dram_tensor("tri2", [128, 2, 128], BF16, kind="ExternalInput")
    y = nc.dram_tensor("y", [S, D], F32, kind="ExternalOutput")

    with tile.TileContext(nc) as tc:
        with (
            tc.tile_pool(name="persist", bufs=1) as persist,
            tc.tile_pool(name="xpool", bufs=32) as xpool,
            tc.tile_pool(name="qk5p", bufs=2) as qk5p,
            tc.tile_pool(name="rot5p", bufs=2) as rot5p,
            tc.tile_pool(name="tp", bufs=2) as tp,
            tc.tile_pool(name="epool", bufs=3) as epool,
            tc.tile_pool(name="recp", bufs=2) as recp,
            tc.tile_pool(name="bcp", bufs=2) as bcp,
            tc.tile_pool(name="ypool", bufs=4) as ypool,
            tc.tile_pool(name="bpsum", bufs=1, space="PSUM") as bpsum,
        ):
            # ---- persistent SBUF ----
            qkT = persist.tile([128, 5, S], BF16, name="qkT")
            vA = persist.tile([128, 16, 65], BF16, name="vA")
            vB = persist.tile([128, 16, 128], BF16, name="vB")
            outT = [persist.tile([128, S], BF16, name=f"outT{t}") for t in range(4)]
            cos_sb = persist.tile([128, S], BF16, name="cos_sb")
            sin_sb = persist.tile([128, S], BF16, name="sin_sb")
            tri_sb = persist.tile([128, 2, 128], BF16, name="tri_sb")
            w6_sb = [persist.tile([128, 768], BF16, name=f"w6_{d}") for d in range(NDT)]
            wo_sb = [persist.tile([128, D], BF16, name=f"wo{f}") for f in range(4)]

            ones64 = persist.tile([128, 64], BF16, name="ones64")
            nc.gpsimd.memset(vB[:], 0.0)
            nc.gpsimd.memset(vA[:, :, 64:65], 1.0)
            nc.gpsimd.memset(vB[:, :, 32:33], 1.0)
            nc.gpsimd.memset(ones64[:], 1.0)

            xt = [[None] * NDT for _ in range(4)]

            def emit_dma_A(sb):
                for d in range(NDT):
                    if sb == 0:
                        nc.sync.dma_start(w6_sb[d][:], w6[d * 128:(d + 1) * 128, :])
                        if d == 4:
                            nc.sync.dma_start(cos_sb[:], cos2[:])
                            nc.sync.dma_start(sin_sb[:], sin2[:])
                        if d == 8:
                            nc.sync.dma_start(tri_sb[:], tri2[:])
                    t_ = xpool.tile([128, 512], BF16, name="xt", tag="xt")
                    nc.sync.dma_start(t_[:], xT[d * 128:(d + 1) * 128,
                                                sb * 512:(sb + 1) * 512])
                    xt[sb][d] = t_

            def a_units(sb, apool):
                """Yield per-unit emit closures for phase A of s-block sb."""
                for hsb in range(2):
                    scol = slice(sb * 512 + hsb * 256, sb * 512 + hsb * 256 + 256)
                    hcol = slice(hsb * 256, hsb * 256 + 256)
                    qk5 = [None]

                    def qk_unit(o, sb=sb, hsb=hsb, hcol=hcol, qk5=qk5):
                        if o == 0:
                            qk5[0] = qk5p.tile([128, 5, 256], BF16, name="qk5",
                                               tag="qk5")
                        ecol = (slice(512, 640) if o == 0
                                else slice((o - 1) * 128, o * 128))
                        acc = apool.tile([128, 512], F32, name="acc", tag="acc")
                        for d in range(NDT):
                            nc.tensor.matmul(acc[:, 0:256], w6_sb[d][:, ecol],
                                             xt[sb][d][:, hcol],
                                             start=(d == 0), stop=(d == NDT - 1))
                        nc.scalar.copy(qk5[0][:, o, :], acc[:, 0:256])

                    def v_unit(u, sb=sb, hsb=hsb):
                        jt = sb * 4 + hsb * 2 + u
                        sc128 = slice(hsb * 256 + u * 128, hsb * 256 + u * 128 + 128)
                        acc = apool.tile([128, 512], F32, name="vacc", tag="acc")
                        for d in range(NDT):
                            nc.tensor.matmul(acc[:, 0:128], xt[sb][d][:, sc128],
                                             w6_sb[d][:, 640:768],
                                             start=(d == 0), stop=(d == NDT - 1))
                        nc.vector.tensor_copy(vA[:, jt, 0:64], acc[:, 0:64])
                        nc.vector.tensor_copy(vB[:, jt, 64:128], acc[:, 64:128])

                    def rope_unit(scol=scol, qk5=qk5):
                        rot5 = rot5p.tile([128, 5, 256], BF16, name="rot5",
                                          tag="rot5")
                        for (a, b_) in ((0, 32), (32, 0), (64, 96), (96, 64)):
                            nc.gpsimd.dma_start(rot5[b_:b_ + 32, :, :],
                                                qk5[0][a:a + 32, :, :])
                        t1 = tp.tile([128, 5, 256], BF16, name="t1", tag="t1")
                        nc.vector.tensor_mul(
                            t1[:], qk5[0][:],
                            cos_sb[:, scol].unsqueeze(1).broadcast_to((128, 5, 256)))
                        t2 = tp.tile([128, 5, 256], BF16, name="t2", tag="t2")
                        nc.vector.tensor_mul(
                            t2[:], rot5[:],
                            sin_sb[:, scol].unsqueeze(1).broadcast_to((128, 5, 256)))
                        nc.vector.tensor_add(qkT[:, :, scol], t1[:], t2[:])

                    for o in range(5):
                        yield lambda o=o, f=qk_unit: f(o)
                    yield lambda f=rope_unit: f()
                    for u in range(2):
                        yield lambda u=u, f=v_unit: f(u)

            def emit_B_unit(bi, t, bcpool, bctag, filler=None):
                icol = slice(bi * 512, (bi + 1) * 512)
                njt = 4 * bi + 4
                fill_at = {3, 7, 11} if bi < 3 else {2, 5, 8, 11}
                pvA = bpsum.tile([65, 512], F32, name="pvA", tag="pvA")
                pvB = bpsum.tile([128, 512], F32, name="pvB", tag="pvB")
                for jt in range(njt):
                    if filler is not None and jt in fill_at:
                        filler()
                    jcol = slice(jt * 128, (jt + 1) * 128)
                    ro = jt - 4 * bi
                    lo = 128 * max(ro, 0)
                    iband = slice(bi * 512 + lo, (bi + 1) * 512)
                    sc = bpsum.tile([128, 2, 512], F32, name="sc", tag="sc",
                                    bufs=2)
                    nc.tensor.matmul(sc[:, 0, lo:], qkT[0:64, 0, jcol],
                                     qkT[0:64, 1 + t, iband],
                                     start=True, stop=True)
                    nc.tensor.matmul(sc[:, 1, lo:], qkT[64:128, 0, jcol],
                                     qkT[64:128, 1 + t, iband],
                                     start=True, stop=True)
                    e = epool.tile([128, 2, 512], BF16, name="e", tag="e")
                    nc.scalar.activation(e[:, :, lo:], sc[:, :, lo:],
                                         mybir.ActivationFunctionType.Exp,
                                         scale=0.125)
                    if ro >= 0:
                        nc.vector.tensor_mul(e[:, :, lo:lo + 128],
                                             e[:, :, lo:lo + 128], tri_sb[:])
                    st, sp = jt == 0, jt == njt - 1
                    nc.tensor.matmul(pvA[:, lo:], vA[:, jt, :], e[:, 0, lo:],
                                     start=st, stop=sp)
                    nc.tensor.matmul(pvB[:, lo:], vB[:, jt, :], e[:, 1, lo:],
                                     start=st, stop=sp)
                # normalization: recip (DVE) now; bc matmuls + muls deferred.
                rec = recp.tile([128, 512], BF16, name="rec", tag="rec")
                with nc.allow_low_precision(reason="bf16 recip, 0.4% ok at 2e-2 tol"):
                    nc.vector.reciprocal(rec[64:65, :], pvA[64:65, :])
                    nc.vector.reciprocal(rec[32:33, :], pvB[32:33, :])

                def tail():
                    bcP1 = bcpool.tile([128, 512], F32, name="bcP1", tag=bctag)
                    nc.tensor.matmul(bcP1[0:64, :], ones64[64:65, :],
                                     rec[64:65, :], start=True, stop=True)
                    bcP2 = bcpool.tile([128, 512], F32, name="bcP2", tag=bctag)
                    nc.tensor.matmul(bcP2[64:128, :], ones64[32:33, :],
                                     rec[32:33, :], start=True, stop=True)
                    bcS = bcp.tile([128, 512], F32, name="bcS", tag="bcS")
                    nc.vector.tensor_copy(bcS[0:64, :], bcP1[0:64, :])
                    nc.vector.tensor_copy(bcS[64:128, :], bcP2[64:128, :])
                    nc.vector.tensor_mul(outT[t][0:64, icol], pvA[0:64, :],
                                         bcS[0:64, :])
                    nc.vector.tensor_mul(outT[t][64:128, icol], pvB[64:128, :],
                                         bcS[64:128, :])
                return tail

            def emit_C_group(stt, db, cpool):
                srow = slice(stt * 128, (stt + 1) * 128)
                dcol = slice(db * 512, (db + 1) * 512)
                yp = cpool.tile([128, 512], F32, name="yp", tag="yp")
                for f in range(4):
                    nc.tensor.matmul(yp[:], outT[f][:, srow],
                                     wo_sb[f][:, dcol],
                                     start=(f == 0), stop=(f == 3))
                ys = ypool.tile([128, 512], F32, name="ys", tag="ys")
                nc.scalar.copy(ys[:], yp[:])
                nc.sync.dma_start(y[srow, dcol], ys[:])

            # ================= emission schedule =================
            from collections import deque

            with tc.tile_pool(name="apsum", bufs=2, space="PSUM") as apool:
                emit_dma_A(0)
                for u in a_units(0, apool):
                    u()
                emit_dma_A(1)
                for u in a_units(1, apool):
                    u()
                emit_dma_A(2)
                afill = deque(a_units(2, apool))
                fa = lambda: afill.popleft()() if afill else None
                for t in range(4):
                    nc.sync.dma_start(wo_sb[t][:], wo[t * 128:(t + 1) * 128, :])
                    tail = emit_B_unit(0, t, apool, "acc", filler=fa)
                    fa()
                    tail()
                    fa()
                emit_dma_A(3)
                afill.extend(a_units(3, apool))
                for t in range(4):
                    tail = emit_B_unit(1, t, apool, "acc", filler=fa)
                    fa()
                    tail()
                    fa()
                while afill:
                    fa()
            with tc.tile_pool(name="cpsum", bufs=2, space="PSUM") as cpool:
                cfill = deque((stt, db) for stt in range(8) for db in range(4))
                fc = lambda: (emit_C_group(*cfill.popleft(), cpool)
                              if cfill else None)
                for t in range(4):
                    tail = emit_B_unit(2, t, cpool, "yp", filler=fc)
                    fc()
                    tail()
                    fc()
                cfill.extend((stt, db) for stt in range(8, 12) for db in range(4))
                for t in range(4):
                    tail = emit_B_unit(3, t, cpool, "yp", filler=fc)
                    fc()
                    tail()
                    fc()
                cfill.extend((stt, db) for stt in range(12, 16) for db in range(4))
                while cfill:
                    fc()

    nc.compile()
    return nc


def host_inputs(x, Wq, Wk, Wv, Wo):
    inv = 1.0 / (10000.0 ** (np.arange(0, HD, 2, dtype=np.float64) / HD))
    freqs = np.outer(np.arange(S, dtype=np.float64), inv)          # [S, 32]
    emb = np.concatenate([freqs, freqs], axis=1)                   # [S, 64]
    cos = np.cos(emb).astype(np.float32)
    sin = np.sin(emb).astype(np.float32)
    cos2 = np.ascontiguousarray(np.tile(cos.T, (2, 1)))            # [128, S]
    sinf = np.concatenate([-sin[:, :32], sin[:, 32:]], axis=1)     # sign-folded
    sin2 = np.ascontiguousarray(np.tile(sinf.T, (2, 1)))
    j = np.arange(128)[:, None, None]
    c = np.arange(128)[None, None, :]
    tri2 = np.broadcast_to(j <= c, (128, 2, 128)).astype(np.float32)

    bf = ml_dtypes.bfloat16
    Wq4 = Wq.reshape(D, H, HD)
    Wo4 = Wo.reshape(H, HD, D)
    Wk4 = Wk.reshape(D, KV, HD)
    Wv4 = Wv.reshape(D, KV, HD)

    maps = []
    for core in range(N_CORES):
        b, g2 = core // 4, core % 4
        gh = [8 * g2 + p for p in PERM]
        w6 = np.concatenate([
            Wq4[:, gh, :].reshape(D, 512),
            Wk4[:, [2 * g2, 2 * g2 + 1], :].reshape(D, 128),
            Wv4[:, [2 * g2, 2 * g2 + 1], :].reshape(D, 128),
        ], axis=1)
        maps.append({
            "xT": np.ascontiguousarray(x[b].T).astype(bf),
            "w6": np.ascontiguousarray(w6).astype(bf),
            "wo": np.ascontiguousarray(Wo4[gh].reshape(512, D)).astype(bf),
            "cos2": cos2.astype(bf), "sin2": sin2.astype(bf),
            "tri2": tri2.astype(bf),
        })
    return maps


_NC_CACHE = None


def kernel(x, Wq, Wk, Wv, Wo):
    global LAST_RESULT, _NC_CACHE
    maps = host_inputs(np.asarray(x, np.float32), np.asarray(Wq, np.float32),
                       np.asarray(Wk, np.float32), np.asarray(Wv, np.float32),
                       np.asarray(Wo, np.float32))
    if _NC_CACHE is None:
        _NC_CACHE = build_nc()
    trace = bool(os.environ.get("KERNEL_TRACE"))
    try:
        res = run_bass_kernel_spmd(_NC_CACHE, maps, list(range(N_CORES)), trace=trace)
    except (ImportError, ModuleNotFoundError):
        res = run_bass_kernel_spmd(_NC_CACHE, maps, list(range(N_CORES)), trace=False)
    LAST_RESULT = res
    out = np.empty((B, S, D), np.float32)
    for b in range(B):
        out[b] = res.results[4 * b]["y"]
        for g2 in range(1, 4):
            out[b] += res.results[4 * b + g2]["y"]
    return out


# revision 3
# speedup vs baseline: 1.3860x; 1.0472x over previous
"""GQA kernel for trn2, 8 NeuronCores — v2 (bf16, phase-interleaved).

Sharding: core c = (b, g2), b = c//4, g2 = c%4: batch b, kv heads {2g2, 2g2+1},
q heads 8g2..8g2+7 permuted [0,4,1,5,2,6,3,7] into 4 pairs t (rows 0:64 = kv0's
q head, 64:128 = kv1's).  Host sums the 4 partial y outputs per batch.

Dataflow (all matmuls bf16 in / f32 psum out):
  A: qkT[e,s] = W.T x per 256-col half-sb, e-output-major (one psum acc ring,
     2 banks); v computed in natural [s,e] orientation (no PE transpose).
     RoPE: ACT drains acc->qk5 (packed [128,5,256] bf16), 4 gpsimd shift-DMAs
     rotate all 5 tensors at once, 3 DVE ops apply cos/sin (broadcast AP).
  B: per (bi,t): fused scores psum [128,2,512] (slot A/B in different banks),
     one fused 3D exp per j-tile, DVE causal mask on diagonal, pv accumulated
     per (bi,t) into pvA[65,512] (rowsumA in row 64) / pvB[128,512] (rowsumB
     in row 32, out rows 64:128 -- partition-aligned with outT, no DMA moves).
     Normalize: DVE reciprocal -> dram round-trip broadcast (sync+gpsimd DGE)
     -> DVE mul psum x bcS -> outT bf16.
  C: y[s,d] = outT.T wo per (stt,db), ACT drain, sync DMA out.

Emission interleaves phases so the in-order PE queue never starves on ACT
(exp) or normalization chains:
  A0 A1 | B0t* x A2-units | B1t* x A3-units | B2t* x C(stt0-3) |
  B3t* x C(stt4-11...) | C tail.
PSUM: acc 2 banks (A, closes after A3) -> yp 2 banks (C); sc 4; pvA 1; pvB 1.
"""

import os
import numpy as np
import ml_dtypes

import concourse.bass as bass
import concourse.bacc as bacc
import concourse.mybir as mybir
import concourse.tile as tile
from concourse.bass_utils import run_bass_kernel_spmd

F32 = mybir.dt.float32
BF16 = mybir.dt.bfloat16

B, S, D = 2, 2048, 2048
H, KV, HD = 32, 8, 64
N_CORES = 8
NDT = D // 128          # 16 d-tiles
NIT = 4                 # 512-wide i-blocks
PERM = [0, 4, 1, 5, 2, 6, 3, 7]

LAST_RESULT = None


def build_nc():
    nc = bacc.Bacc("TRN2", target_bir_lowering=False, debug=False,
                   enable_asserts=True, num_devices=N_CORES)

    xT = nc.dram_tensor("xT", [D, S], BF16, kind="ExternalInput")
    w6 = nc.dram_tensor("w6", [D, 768], BF16, kind="ExternalInput")
    wo = nc.dram_tensor("wo", [512, D], BF16, kind="ExternalInput")
    cos2 = nc.dram_tensor("cos2", [128, S], BF16, kind="ExternalInput")
    sin2 = nc.dram_tensor("sin2", [128, S], BF16, kind="ExternalInput")
    tri2 = nc.dram_tensor("tri2", [128, 2, 128], BF16, kind="ExternalInput")
    y = nc.dram_tensor("y", [S, D], F32, kind="ExternalOutput")

    with tile.TileContext(nc) as tc:
        with (
            tc.tile_pool(name="persist", bufs=1) as persist,
            tc.tile_pool(name="xpool", bufs=32) as xpool,
            tc.tile_pool(name="qk5p", bufs=2) as qk5p,
            tc.tile_pool(name="rot5p", bufs=2) as rot5p,
            tc.tile_pool(name="tp", bufs=2) as tp,
            tc.tile_pool(name="epool", bufs=3) as epool,
            tc.tile_pool(name="recp", bufs=2) as recp,
            tc.tile_pool(name="bcp", bufs=2) as bcp,
            tc.tile_pool(name="ypool", bufs=4) as ypool,
            tc.tile_pool(name="bpsum", bufs=1, space="PSUM") as bpsum,
        ):
            # ---- persistent SBUF ----
            qkT = persist.tile([128, 5, S], BF16, name="qkT")
            vA = persist.tile([128, 16, 65], BF16, name="vA")
            vB = persist.tile([128, 16, 128], BF16, name="vB")
            outT = [persist.tile([128, S], BF16, name=f"outT{t}") for t in range(4)]
            cos_sb = persist.tile([128, S], BF16, name="cos_sb")
            sin_sb = persist.tile([128, S], BF16, name="sin_sb")
            tri_sb = persist.tile([128, 2, 128], BF16, name="tri_sb")
            w6_sb = [persist.tile([128, 768], BF16, name=f"w6_{d}") for d in range(NDT)]
            wo_sb = [persist.tile([128, D], BF16, name=f"wo{f}") for f in range(4)]

            ones64 = persist.tile([128, 64], BF16, name="ones64")
            nc.gpsimd.memset(vB[:], 0.0)
            nc.gpsimd.memset(vA[:, :, 64:65], 1.0)
            nc.gpsimd.memset(vB[:, :, 32:33], 1.0)
            nc.gpsimd.memset(ones64[:], 1.0)

            xt = [[None] * NDT for _ in range(4)]

            def emit_dma_A(sb):
                # sb0: alternate the two DGE queues so the d-tile stream
                # arrives ~2x faster during the cold start.
                for d in range(NDT):
                    if sb == 0:
                        eng = nc.sync if d % 2 == 0 else nc.gpsimd
                        eng.dma_start(w6_sb[d][:], w6[d * 128:(d + 1) * 128, :])
                        if d == 4:
                            nc.sync.dma_start(cos_sb[:], cos2[:])
                            nc.sync.dma_start(sin_sb[:], sin2[:])
                        if d == 8:
                            nc.sync.dma_start(tri_sb[:], tri2[:])
                    t_ = xpool.tile([128, 512], BF16, name="xt", tag="xt")
                    eng = nc.gpsimd if (sb == 0 and d % 2 == 0) else nc.sync
                    eng.dma_start(t_[:], xT[d * 128:(d + 1) * 128,
                                            sb * 512:(sb + 1) * 512])
                    xt[sb][d] = t_

            def emit_A0_douter(apool):
                """d-outer accumulation for sb0/h0: consume each arriving
                x d-tile across all 7 outputs at once, borrowing the idle
                sc/pvB banks for extra accumulators."""
                qk5 = qk5p.tile([128, 5, 256], BF16, name="qk5", tag="qk5")
                accK = apool.tile([128, 512], F32, name="acc", tag="acc")
                accQ0 = apool.tile([128, 512], F32, name="acc", tag="acc")
                scT1 = bpsum.tile([128, 2, 512], F32, name="sc", tag="sc", bufs=2)
                scT2 = bpsum.tile([128, 2, 512], F32, name="sc", tag="sc", bufs=2)
                pvBT = bpsum.tile([128, 512], F32, name="pvB", tag="pvB")
                qk_dst = [accK[:, 0:256], accQ0[:, 0:256], scT1[:, 0, 0:256],
                          scT1[:, 1, 0:256], scT2[:, 0, 0:256]]
                for d in range(NDT):
                    st, sp = d == 0, d == NDT - 1
                    for o in range(5):
                        ecol = (slice(512, 640) if o == 0
                                else slice((o - 1) * 128, o * 128))
                        nc.tensor.matmul(qk_dst[o], w6_sb[d][:, ecol],
                                         xt[0][d][:, 0:256], start=st, stop=sp)
                    nc.tensor.matmul(scT2[:, 1, 0:128], xt[0][d][:, 0:128],
                                     w6_sb[d][:, 640:768], start=st, stop=sp)
                    nc.tensor.matmul(pvBT[:, 0:128], xt[0][d][:, 128:256],
                                     w6_sb[d][:, 640:768], start=st, stop=sp)
                for o in range(5):
                    nc.scalar.copy(qk5[:, o, :], qk_dst[o])
                nc.vector.tensor_copy(vA[:, 0, 0:64], scT2[:, 1, 0:64])
                nc.vector.tensor_copy(vB[:, 0, 64:128], scT2[:, 1, 64:128])
                nc.vector.tensor_copy(vA[:, 1, 0:64], pvBT[:, 0:64])
                nc.vector.tensor_copy(vB[:, 1, 64:128], pvBT[:, 64:128])
                rot5 = rot5p.tile([128, 5, 256], BF16, name="rot5", tag="rot5")
                for (a, b_) in ((0, 32), (32, 0), (64, 96), (96, 64)):
                    nc.gpsimd.dma_start(rot5[b_:b_ + 32, :, :],
                                        qk5[a:a + 32, :, :])
                t1 = tp.tile([128, 5, 256], BF16, name="t1", tag="t1")
                nc.vector.tensor_mul(
                    t1[:], qk5[:],
                    cos_sb[:, 0:256].unsqueeze(1).broadcast_to((128, 5, 256)))
                t2 = tp.tile([128, 5, 256], BF16, name="t2", tag="t2")
                nc.vector.tensor_mul(
                    t2[:], rot5[:],
                    sin_sb[:, 0:256].unsqueeze(1).broadcast_to((128, 5, 256)))
                nc.vector.tensor_add(qkT[:, :, 0:256], t1[:], t2[:])

            def a_units(sb, apool, hsbs=(0, 1)):
                """Yield per-unit emit closures for phase A of s-block sb."""
                for hsb in hsbs:
                    scol = slice(sb * 512 + hsb * 256, sb * 512 + hsb * 256 + 256)
                    hcol = slice(hsb * 256, hsb * 256 + 256)
                    qk5 = [None]

                    def qk_unit(o, sb=sb, hsb=hsb, hcol=hcol, qk5=qk5):
                        if o == 0:
                            qk5[0] = qk5p.tile([128, 5, 256], BF16, name="qk5",
                                               tag="qk5")
                        ecol = (slice(512, 640) if o == 0
                                else slice((o - 1) * 128, o * 128))
                        acc = apool.tile([128, 512], F32, name="acc", tag="acc")
                        for d in range(NDT):
                            nc.tensor.matmul(acc[:, 0:256], w6_sb[d][:, ecol],
                                             xt[sb][d][:, hcol],
                                             start=(d == 0), stop=(d == NDT - 1))
                        nc.scalar.copy(qk5[0][:, o, :], acc[:, 0:256])

                    def v_unit(u, sb=sb, hsb=hsb):
                        jt = sb * 4 + hsb * 2 + u
                        sc128 = slice(hsb * 256 + u * 128, hsb * 256 + u * 128 + 128)
                        acc = apool.tile([128, 512], F32, name="vacc", tag="acc")
                        for d in range(NDT):
                            nc.tensor.matmul(acc[:, 0:128], xt[sb][d][:, sc128],
                                             w6_sb[d][:, 640:768],
                                             start=(d == 0), stop=(d == NDT - 1))
                        nc.vector.tensor_copy(vA[:, jt, 0:64], acc[:, 0:64])
                        nc.vector.tensor_copy(vB[:, jt, 64:128], acc[:, 64:128])

                    def rope_unit(scol=scol, qk5=qk5):
                        rot5 = rot5p.tile([128, 5, 256], BF16, name="rot5",
                                          tag="rot5")
                        for (a, b_) in ((0, 32), (32, 0), (64, 96), (96, 64)):
                            nc.gpsimd.dma_start(rot5[b_:b_ + 32, :, :],
                                                qk5[0][a:a + 32, :, :])
                        t1 = tp.tile([128, 5, 256], BF16, name="t1", tag="t1")
                        nc.vector.tensor_mul(
                            t1[:], qk5[0][:],
                            cos_sb[:, scol].unsqueeze(1).broadcast_to((128, 5, 256)))
                        t2 = tp.tile([128, 5, 256], BF16, name="t2", tag="t2")
                        nc.vector.tensor_mul(
                            t2[:], rot5[:],
                            sin_sb[:, scol].unsqueeze(1).broadcast_to((128, 5, 256)))
                        nc.vector.tensor_add(qkT[:, :, scol], t1[:], t2[:])

                    for o in range(5):
                        yield lambda o=o, f=qk_unit: f(o)
                    yield lambda f=rope_unit: f()
                    for u in range(2):
                        yield lambda u=u, f=v_unit: f(u)

            def emit_B_unit(bi, t, bcpool, bctag, filler=None):
                icol = slice(bi * 512, (bi + 1) * 512)
                njt = 4 * bi + 4
                fill_at = {3, 7, 11} if bi < 3 else {2, 5, 8, 11}
                pvA = bpsum.tile([65, 512], F32, name="pvA", tag="pvA")
                pvB = bpsum.tile([128, 512], F32, name="pvB", tag="pvB")
                for jt in range(njt):
                    if filler is not None and jt in fill_at:
                        filler()
                    jcol = slice(jt * 128, (jt + 1) * 128)
                    ro = jt - 4 * bi
                    lo = 128 * max(ro, 0)
                    iband = slice(bi * 512 + lo, (bi + 1) * 512)
                    sc = bpsum.tile([128, 2, 512], F32, name="sc", tag="sc",
                                    bufs=2)
                    nc.tensor.matmul(sc[:, 0, lo:], qkT[0:64, 0, jcol],
                                     qkT[0:64, 1 + t, iband],
                                     start=True, stop=True)
                    nc.tensor.matmul(sc[:, 1, lo:], qkT[64:128, 0, jcol],
                                     qkT[64:128, 1 + t, iband],
                                     start=True, stop=True)
                    e = epool.tile([128, 2, 512], BF16, name="e", tag="e")
                    nc.scalar.activation(e[:, :, lo:], sc[:, :, lo:],
                                         mybir.ActivationFunctionType.Exp,
                                         scale=0.125)
                    if ro >= 0:
                        nc.vector.tensor_mul(e[:, :, lo:lo + 128],
                                             e[:, :, lo:lo + 128], tri_sb[:])
                    st, sp = jt == 0, jt == njt - 1
                    nc.tensor.matmul(pvA[:, lo:], vA[:, jt, :], e[:, 0, lo:],
                                     start=st, stop=sp)
                    nc.tensor.matmul(pvB[:, lo:], vB[:, jt, :], e[:, 1, lo:],
                                     start=st, stop=sp)
                # normalization: recip (DVE) now; bc matmuls + muls deferred.
                rec = recp.tile([128, 512], BF16, name="rec", tag="rec")
                with nc.allow_low_precision(reason="bf16 recip, 0.4% ok at 2e-2 tol"):
                    nc.vector.reciprocal(rec[64:65, :], pvA[64:65, :])
                    nc.vector.reciprocal(rec[32:33, :], pvB[32:33, :])

                def tail():
                    bcP1 = bcpool.tile([128, 512], F32, name="bcP1", tag=bctag)
                    nc.tensor.matmul(bcP1[0:64, :], ones64[64:65, :],
                                     rec[64:65, :], start=True, stop=True)
                    bcP2 = bcpool.tile([128, 512], F32, name="bcP2", tag=bctag)
                    nc.tensor.matmul(bcP2[64:128, :], ones64[32:33, :],
                                     rec[32:33, :], start=True, stop=True)
                    bcS = bcp.tile([128, 512], F32, name="bcS", tag="bcS")
                    nc.vector.tensor_copy(bcS[0:64, :], bcP1[0:64, :])
                    nc.vector.tensor_copy(bcS[64:128, :], bcP2[64:128, :])
                    nc.vector.tensor_mul(outT[t][0:64, icol], pvA[0:64, :],
                                         bcS[0:64, :])
                    nc.vector.tensor_mul(outT[t][64:128, icol], pvB[64:128, :],
                                         bcS[64:128, :])
                return tail

            def emit_C_group(stt, db, cpool):
                srow = slice(stt * 128, (stt + 1) * 128)
                dcol = slice(db * 512, (db + 1) * 512)
                yp = cpool.tile([128, 512], F32, name="yp", tag="yp")
                for f in range(4):
                    nc.tensor.matmul(yp[:], outT[f][:, srow],
                                     wo_sb[f][:, dcol],
                                     start=(f == 0), stop=(f == 3))
                ys = ypool.tile([128, 512], F32, name="ys", tag="ys")
                nc.vector.tensor_copy(ys[:], yp[:])
                nc.sync.dma_start(y[srow, dcol], ys[:])

            # ================= emission schedule =================
            from collections import deque

            with tc.tile_pool(name="apsum", bufs=2, space="PSUM") as apool:
                emit_dma_A(0)
                emit_A0_douter(apool)
                for u in a_units(0, apool, hsbs=(1,)):
                    u()
                emit_dma_A(1)
                for u in a_units(1, apool):
                    u()
                emit_dma_A(2)
                afill = deque(a_units(2, apool))
                fa = lambda: afill.popleft()() if afill else None
                for t in range(4):
                    nc.sync.dma_start(wo_sb[t][:], wo[t * 128:(t + 1) * 128, :])
                    tail = emit_B_unit(0, t, apool, "acc", filler=fa)
                    fa()
                    tail()
                    fa()
                emit_dma_A(3)
                afill.extend(a_units(3, apool))
                for t in range(4):
                    tail = emit_B_unit(1, t, apool, "acc", filler=fa)
                    fa()
                    tail()
                    fa()
                while afill:
                    fa()
            with tc.tile_pool(name="cpsum", bufs=2, space="PSUM") as cpool:
                cfill = deque((stt, db) for stt in range(8) for db in range(4))
                fc = lambda: (emit_C_group(*cfill.popleft(), cpool)
                              if cfill else None)
                for t in range(4):
                    tail = emit_B_unit(2, t, cpool, "yp", filler=fc)
                    fc()
                    fc()
                    tail()
                    fc()
                cfill.extend((stt, db) for stt in range(8, 12) for db in range(4))
                for t in range(4):
                    tail = emit_B_unit(3, t, cpool, "yp", filler=fc)
                    fc()
                    fc()
                    tail()
                    fc()
                cfill.extend((stt, db) for stt in range(12, 16) for db in range(4))
                while cfill:
                    fc()

    nc.compile()
    return nc


def host_inputs(x, Wq, Wk, Wv, Wo):
    inv = 1.0 / (10000.0 ** (np.arange(0, HD, 2, dtype=np.float64) / HD))
    freqs = np.outer(np.arange(S, dtype=np.float64), inv)          # [S, 32]
    emb = np.concatenate([freqs, freqs], axis=1)                   # [S, 64]
    cos = np.cos(emb).astype(np.float32)
    sin = np.sin(emb).astype(np.float32)
    cos2 = np.ascontiguousarray(np.tile(cos.T, (2, 1)))            # [128, S]
    sinf = np.concatenate([-sin[:, :32], sin[:, 32:]], axis=1)     # sign-folded
    sin2 = np.ascontiguousarray(np.tile(sinf.T, (2, 1)))
    j = np.arange(128)[:, None, None]
    c = np.arange(128)[None, None, :]
    tri2 = np.broadcast_to(j <= c, (128, 2, 128)).astype(np.float32)

    bf = ml_dtypes.bfloat16
    Wq4 = Wq.reshape(D, H, HD)
    Wo4 = Wo.reshape(H, HD, D)
    Wk4 = Wk.reshape(D, KV, HD)
    Wv4 = Wv.reshape(D, KV, HD)

    maps = []
    for core in range(N_CORES):
        b, g2 = core // 4, core % 4
        gh = [8 * g2 + p for p in PERM]
        w6 = np.concatenate([
            Wq4[:, gh, :].reshape(D, 512),
            Wk4[:, [2 * g2, 2 * g2 + 1], :].reshape(D, 128),
            Wv4[:, [2 * g2, 2 * g2 + 1], :].reshape(D, 128),
        ], axis=1)
        maps.append({
            "xT": np.ascontiguousarray(x[b].T).astype(bf),
            "w6": np.ascontiguousarray(w6).astype(bf),
            "wo": np.ascontiguousarray(Wo4[gh].reshape(512, D)).astype(bf),
            "cos2": cos2.astype(bf), "sin2": sin2.astype(bf),
            "tri2": tri2.astype(bf),
        })
    return maps


_NC_CACHE = None


def kernel(x, Wq, Wk, Wv, Wo):
    global LAST_RESULT, _NC_CACHE
    maps = host_inputs(np.asarray(x, np.float32), np.asarray(Wq, np.float32),
                       np.asarray(Wk, np.float32), np.asarray(Wv, np.float32),
                       np.asarray(Wo, np.float32))
    if _NC_CACHE is None:
        _NC_CACHE = build_nc()
    trace = bool(os.environ.get("KERNEL_TRACE"))
    try:
        res = run_bass_kernel_spmd(_NC_CACHE, maps, list(range(N_CORES)), trace=trace)
    except (ImportError, ModuleNotFoundError):
        res = run_bass_kernel_spmd(_NC_CACHE, maps, list(range(N_CORES)), trace=False)
    LAST_RESULT = res
    out = np.empty((B, S, D), np.float32)
    for b in range(B):
        out[b] = res.results[4 * b]["y"]
        for g2 in range(1, 4):
            out[b] += res.results[4 * b + g2]["y"]
    return out


# revision 4
# speedup vs baseline: 1.4130x; 1.0194x over previous
"""GQA kernel for trn2, 8 NeuronCores — v2 (bf16, phase-interleaved).

Sharding: core c = (b, g2), b = c//4, g2 = c%4: batch b, kv heads {2g2, 2g2+1},
q heads 8g2..8g2+7 permuted [0,4,1,5,2,6,3,7] into 4 pairs t (rows 0:64 = kv0's
q head, 64:128 = kv1's).  Host sums the 4 partial y outputs per batch.

Dataflow (all matmuls bf16 in / f32 psum out):
  A: qkT[e,s] = W.T x per 256-col half-sb, e-output-major (one psum acc ring,
     2 banks); v computed in natural [s,e] orientation (no PE transpose).
     RoPE: ACT drains acc->qk5 (packed [128,5,256] bf16), 4 gpsimd shift-DMAs
     rotate all 5 tensors at once, 3 DVE ops apply cos/sin (broadcast AP).
  B: per (bi,t): fused scores psum [128,2,512] (slot A/B in different banks),
     one fused 3D exp per j-tile, DVE causal mask on diagonal, pv accumulated
     per (bi,t) into pvA[65,512] (rowsumA in row 64) / pvB[128,512] (rowsumB
     in row 32, out rows 64:128 -- partition-aligned with outT, no DMA moves).
     Normalize: DVE reciprocal -> dram round-trip broadcast (sync+gpsimd DGE)
     -> DVE mul psum x bcS -> outT bf16.
  C: y[s,d] = outT.T wo per (stt,db), ACT drain, sync DMA out.

Emission interleaves phases so the in-order PE queue never starves on ACT
(exp) or normalization chains:
  A0 A1 | B0t* x A2-units | B1t* x A3-units | B2t* x C(stt0-3) |
  B3t* x C(stt4-11...) | C tail.
PSUM: acc 2 banks (A, closes after A3) -> yp 2 banks (C); sc 4; pvA 1; pvB 1.
"""

import os
import numpy as np
import ml_dtypes

import concourse.bass as bass
import concourse.bacc as bacc
import concourse.mybir as mybir
import concourse.tile as tile
from concourse.bass_utils import run_bass_kernel_spmd

F32 = mybir.dt.float32
BF16 = mybir.dt.bfloat16

B, S, D = 2, 2048, 2048
H, KV, HD = 32, 8, 64
N_CORES = 8
NDT = D // 128          # 16 d-tiles
NIT = 4                 # 512-wide i-blocks
PERM = [0, 4, 1, 5, 2, 6, 3, 7]

LAST_RESULT = None


def build_nc():
    nc = bacc.Bacc("TRN2", target_bir_lowering=False, debug=False,
                   enable_asserts=True, num_devices=N_CORES)

    xT = nc.dram_tensor("xT", [D, S], BF16, kind="ExternalInput")
    w6 = nc.dram_tensor("w6", [D, 768], BF16, kind="ExternalInput")
    wo = nc.dram_tensor("wo", [512, D], BF16, kind="ExternalInput")
    cos2 = nc.dram_tensor("cos2", [128, S], BF16, kind="ExternalInput")
    sin2 = nc.dram_tensor("sin2", [128, S], BF16, kind="ExternalInput")
    tri2 = nc.dram_tensor("tri2", [128, 2, 128], BF16, kind="ExternalInput")
    y = nc.dram_tensor("y", [S, D], F32, kind="ExternalOutput")

    with tile.TileContext(nc) as tc:
        with (
            tc.tile_pool(name="persist", bufs=1) as persist,
            tc.tile_pool(name="xpool", bufs=32) as xpool,
            tc.tile_pool(name="qk5p", bufs=2) as qk5p,
            tc.tile_pool(name="rot5p", bufs=2) as rot5p,
            tc.tile_pool(name="tp", bufs=2) as tp,
            tc.tile_pool(name="epool", bufs=4) as epool,
            tc.tile_pool(name="recp", bufs=2) as recp,
            tc.tile_pool(name="bcp", bufs=2) as bcp,
            tc.tile_pool(name="ypool", bufs=4) as ypool,
            tc.tile_pool(name="bpsum", bufs=1, space="PSUM") as bpsum,
        ):
            # ---- persistent SBUF ----
            qkT = persist.tile([128, 5, S], BF16, name="qkT")
            vA = persist.tile([128, 16, 65], BF16, name="vA")
            vB = persist.tile([128, 16, 128], BF16, name="vB")
            outT = [persist.tile([128, S], BF16, name=f"outT{t}") for t in range(4)]
            cos_sb = persist.tile([128, S], BF16, name="cos_sb")
            sin_sb = persist.tile([128, S], BF16, name="sin_sb")
            tri_sb = persist.tile([128, 2, 128], BF16, name="tri_sb")
            w6_sb = [persist.tile([128, 768], BF16, name=f"w6_{d}") for d in range(NDT)]
            wo_sb = [persist.tile([128, D], BF16, name=f"wo{f}") for f in range(4)]

            ones64 = persist.tile([128, 64], BF16, name="ones64")
            nc.gpsimd.memset(vB[:], 0.0)
            nc.gpsimd.memset(vA[:, :, 64:65], 1.0)
            nc.gpsimd.memset(vB[:, :, 32:33], 1.0)
            nc.gpsimd.memset(ones64[:], 1.0)

            xt = [[None] * NDT for _ in range(4)]

            def emit_dma_A(sb):
                # sb0: alternate the two DGE queues so the d-tile stream
                # arrives ~2x faster during the cold start.
                for d in range(NDT):
                    if sb == 0:
                        eng = nc.sync if d % 2 == 0 else nc.gpsimd
                        eng.dma_start(w6_sb[d][:], w6[d * 128:(d + 1) * 128, :])
                        if d == 4:
                            nc.sync.dma_start(cos_sb[:], cos2[:])
                            nc.sync.dma_start(sin_sb[:], sin2[:])
                        if d == 8:
                            nc.sync.dma_start(tri_sb[:], tri2[:])
                    t_ = xpool.tile([128, 512], BF16, name="xt", tag="xt")
                    eng = nc.gpsimd if (sb == 0 and d % 2 == 0) else nc.sync
                    eng.dma_start(t_[:], xT[d * 128:(d + 1) * 128,
                                            sb * 512:(sb + 1) * 512])
                    xt[sb][d] = t_

            def emit_A0_douter(apool):
                """Full-width d-outer qk accumulation for sb0: consume each
                arriving x d-tile across all 5 qk outputs (512 cols each) so
                PE keeps pace with the serialized cold-start DMA stream.
                Borrows the idle sc banks for three extra accumulators."""
                accK = apool.tile([128, 512], F32, name="acc", tag="acc")
                accQ0 = apool.tile([128, 512], F32, name="acc", tag="acc")
                scT1 = bpsum.tile([128, 2, 512], F32, name="sc", tag="sc", bufs=2)
                scT2 = bpsum.tile([128, 2, 512], F32, name="sc", tag="sc", bufs=2)
                qk_dst = [accK[:, :], accQ0[:, :], scT1[:, 0, :], scT1[:, 1, :],
                          scT2[:, 0, :]]
                for d in range(NDT):
                    st, sp = d == 0, d == NDT - 1
                    for o in range(5):
                        ecol = (slice(512, 640) if o == 0
                                else slice((o - 1) * 128, o * 128))
                        nc.tensor.matmul(qk_dst[o], w6_sb[d][:, ecol],
                                         xt[0][d][:, 0:512], start=st, stop=sp)
                for hsb in range(2):
                    scol = slice(hsb * 256, hsb * 256 + 256)
                    qk5 = qk5p.tile([128, 5, 256], BF16, name="qk5", tag="qk5")
                    for o in range(5):
                        nc.scalar.copy(qk5[:, o, :], qk_dst[o][:, scol])
                    rot5 = rot5p.tile([128, 5, 256], BF16, name="rot5",
                                      tag="rot5")
                    for (a, b_) in ((0, 32), (32, 0), (64, 96), (96, 64)):
                        nc.gpsimd.dma_start(rot5[b_:b_ + 32, :, :],
                                            qk5[a:a + 32, :, :])
                    t1 = tp.tile([128, 5, 256], BF16, name="t1", tag="t1")
                    nc.vector.tensor_mul(
                        t1[:], qk5[:],
                        cos_sb[:, scol].unsqueeze(1).broadcast_to((128, 5, 256)))
                    t2 = tp.tile([128, 5, 256], BF16, name="t2", tag="t2")
                    nc.vector.tensor_mul(
                        t2[:], rot5[:],
                        sin_sb[:, scol].unsqueeze(1).broadcast_to((128, 5, 256)))
                    nc.vector.tensor_add(qkT[:, :, scol], t1[:], t2[:])
                for u in range(4):
                    acc = apool.tile([128, 512], F32, name="vacc", tag="acc")
                    for d in range(NDT):
                        nc.tensor.matmul(acc[:, 0:128],
                                         xt[0][d][:, u * 128:(u + 1) * 128],
                                         w6_sb[d][:, 640:768],
                                         start=(d == 0), stop=(d == NDT - 1))
                    nc.vector.tensor_copy(vA[:, u, 0:64], acc[:, 0:64])
                    nc.vector.tensor_copy(vB[:, u, 64:128], acc[:, 64:128])

            def a_units(sb, apool, hsbs=(0, 1)):
                """Yield per-unit emit closures for phase A of s-block sb."""
                for hsb in hsbs:
                    scol = slice(sb * 512 + hsb * 256, sb * 512 + hsb * 256 + 256)
                    hcol = slice(hsb * 256, hsb * 256 + 256)
                    qk5 = [None]

                    def qk_unit(o, sb=sb, hsb=hsb, hcol=hcol, qk5=qk5):
                        if o == 0:
                            qk5[0] = qk5p.tile([128, 5, 256], BF16, name="qk5",
                                               tag="qk5")
                        ecol = (slice(512, 640) if o == 0
                                else slice((o - 1) * 128, o * 128))
                        acc = apool.tile([128, 512], F32, name="acc", tag="acc")
                        for d in range(NDT):
                            nc.tensor.matmul(acc[:, 0:256], w6_sb[d][:, ecol],
                                             xt[sb][d][:, hcol],
                                             start=(d == 0), stop=(d == NDT - 1))
                        nc.scalar.copy(qk5[0][:, o, :], acc[:, 0:256])

                    def v_unit(u, sb=sb, hsb=hsb):
                        jt = sb * 4 + hsb * 2 + u
                        sc128 = slice(hsb * 256 + u * 128, hsb * 256 + u * 128 + 128)
                        acc = apool.tile([128, 512], F32, name="vacc", tag="acc")
                        for d in range(NDT):
                            nc.tensor.matmul(acc[:, 0:128], xt[sb][d][:, sc128],
                                             w6_sb[d][:, 640:768],
                                             start=(d == 0), stop=(d == NDT - 1))
                        nc.vector.tensor_copy(vA[:, jt, 0:64], acc[:, 0:64])
                        nc.vector.tensor_copy(vB[:, jt, 64:128], acc[:, 64:128])

                    def rope_unit(scol=scol, qk5=qk5):
                        rot5 = rot5p.tile([128, 5, 256], BF16, name="rot5",
                                          tag="rot5")
                        for (a, b_) in ((0, 32), (32, 0), (64, 96), (96, 64)):
                            nc.gpsimd.dma_start(rot5[b_:b_ + 32, :, :],
                                                qk5[0][a:a + 32, :, :])
                        t1 = tp.tile([128, 5, 256], BF16, name="t1", tag="t1")
                        nc.vector.tensor_mul(
                            t1[:], qk5[0][:],
                            cos_sb[:, scol].unsqueeze(1).broadcast_to((128, 5, 256)))
                        t2 = tp.tile([128, 5, 256], BF16, name="t2", tag="t2")
                        nc.vector.tensor_mul(
                            t2[:], rot5[:],
                            sin_sb[:, scol].unsqueeze(1).broadcast_to((128, 5, 256)))
                        nc.vector.tensor_add(qkT[:, :, scol], t1[:], t2[:])

                    for o in range(5):
                        yield lambda o=o, f=qk_unit: f(o)
                    yield lambda f=rope_unit: f()
                    for u in range(2):
                        yield lambda u=u, f=v_unit: f(u)

            def emit_B_unit(bi, t, bcpool, bctag, filler=None):
                icol = slice(bi * 512, (bi + 1) * 512)
                njt = 4 * bi + 4
                fill_at = ({3, 7} if bi == 2 else
                           {2, 5, 8, 11, 14} if bi == 3 else {3, 7, 11})
                pvA = bpsum.tile([65, 512], F32, name="pvA", tag="pvA")
                pvB = bpsum.tile([128, 512], F32, name="pvB", tag="pvB")
                for jt in range(njt):
                    if filler is not None and jt in fill_at:
                        filler()
                    jcol = slice(jt * 128, (jt + 1) * 128)
                    ro = jt - 4 * bi
                    lo = 128 * max(ro, 0)
                    iband = slice(bi * 512 + lo, (bi + 1) * 512)
                    sc = bpsum.tile([128, 2, 512], F32, name="sc", tag="sc",
                                    bufs=2)
                    nc.tensor.matmul(sc[:, 0, lo:], qkT[0:64, 0, jcol],
                                     qkT[0:64, 1 + t, iband],
                                     start=True, stop=True)
                    nc.tensor.matmul(sc[:, 1, lo:], qkT[64:128, 0, jcol],
                                     qkT[64:128, 1 + t, iband],
                                     start=True, stop=True)
                    e = epool.tile([128, 2, 512], BF16, name="e", tag="e")
                    nc.scalar.activation(e[:, :, lo:], sc[:, :, lo:],
                                         mybir.ActivationFunctionType.Exp,
                                         scale=0.125)
                    if ro >= 0:
                        nc.vector.tensor_mul(e[:, :, lo:lo + 128],
                                             e[:, :, lo:lo + 128], tri_sb[:])
                    st, sp = jt == 0, jt == njt - 1
                    nc.tensor.matmul(pvA[:, lo:], vA[:, jt, :], e[:, 0, lo:],
                                     start=st, stop=sp)
                    nc.tensor.matmul(pvB[:, lo:], vB[:, jt, :], e[:, 1, lo:],
                                     start=st, stop=sp)
                # normalization: recip (DVE) now; bc matmuls + muls deferred.
                rec = recp.tile([128, 512], BF16, name="rec", tag="rec")
                with nc.allow_low_precision(reason="bf16 recip, 0.4% ok at 2e-2 tol"):
                    nc.vector.reciprocal(rec[64:65, :], pvA[64:65, :])
                    nc.vector.reciprocal(rec[32:33, :], pvB[32:33, :])

                def tail():
                    bcP1 = bcpool.tile([128, 512], F32, name="bcP1", tag=bctag)
                    nc.tensor.matmul(bcP1[0:64, :], ones64[64:65, :],
                                     rec[64:65, :], start=True, stop=True)
                    bcP2 = bcpool.tile([128, 512], F32, name="bcP2", tag=bctag)
                    nc.tensor.matmul(bcP2[64:128, :], ones64[32:33, :],
                                     rec[32:33, :], start=True, stop=True)
                    bcS = bcp.tile([128, 512], F32, name="bcS", tag="bcS")
                    nc.vector.tensor_copy(bcS[0:64, :], bcP1[0:64, :])
                    nc.vector.tensor_copy(bcS[64:128, :], bcP2[64:128, :])
                    nc.vector.tensor_mul(outT[t][0:64, icol], pvA[0:64, :],
                                         bcS[0:64, :])
                    nc.vector.tensor_mul(outT[t][64:128, icol], pvB[64:128, :],
                                         bcS[64:128, :])
                return tail

            def emit_C_group(stt, db, cpool):
                srow = slice(stt * 128, (stt + 1) * 128)
                dcol = slice(db * 512, (db + 1) * 512)
                yp = cpool.tile([128, 512], F32, name="yp", tag="yp")
                for f in range(4):
                    nc.tensor.matmul(yp[:], outT[f][:, srow],
                                     wo_sb[f][:, dcol],
                                     start=(f == 0), stop=(f == 3))
                ys = ypool.tile([128, 512], F32, name="ys", tag="ys")
                nc.vector.tensor_copy(ys[:], yp[:])
                nc.sync.dma_start(y[srow, dcol], ys[:])

            # ================= emission schedule =================
            from collections import deque

            with tc.tile_pool(name="apsum", bufs=2, space="PSUM") as apool:
                emit_dma_A(0)
                emit_A0_douter(apool)
                emit_dma_A(1)
                for u in a_units(1, apool):
                    u()
                emit_dma_A(2)
                afill = deque(a_units(2, apool))
                fa = lambda: afill.popleft()() if afill else None
                for t in range(4):
                    nc.sync.dma_start(wo_sb[t][:], wo[t * 128:(t + 1) * 128, :])
                    tail = emit_B_unit(0, t, apool, "acc", filler=fa)
                    fa()
                    tail()
                    fa()
                emit_dma_A(3)
                afill.extend(a_units(3, apool))
                for t in range(4):
                    tail = emit_B_unit(1, t, apool, "acc", filler=fa)
                    fa()
                    tail()
                    fa()
                while afill:
                    fa()
            with tc.tile_pool(name="cpsum", bufs=2, space="PSUM") as cpool:
                cfill = deque((stt, db) for stt in range(8) for db in range(4))
                fc = lambda: (emit_C_group(*cfill.popleft(), cpool)
                              if cfill else None)
                for t in range(4):
                    tail = emit_B_unit(2, t, cpool, "yp", filler=fc)
                    fc()
                    fc()
                    tail()
                    fc()
                cfill.extend((stt, db) for stt in range(8, 12) for db in range(4))
                for t in range(4):
                    tail = emit_B_unit(3, t, cpool, "yp", filler=fc)
                    fc()
                    tail()
                    fc()
                cfill.extend((stt, db) for stt in range(12, 16) for db in range(4))
                while cfill:
                    fc()

    nc.compile()
    return nc


def host_inputs(x, Wq, Wk, Wv, Wo):
    inv = 1.0 / (10000.0 ** (np.arange(0, HD, 2, dtype=np.float64) / HD))
    freqs = np.outer(np.arange(S, dtype=np.float64), inv)          # [S, 32]
    emb = np.concatenate([freqs, freqs], axis=1)                   # [S, 64]
    cos = np.cos(emb).astype(np.float32)
    sin = np.sin(emb).astype(np.float32)
    cos2 = np.ascontiguousarray(np.tile(cos.T, (2, 1)))            # [128, S]
    sinf = np.concatenate([-sin[:, :32], sin[:, 32:]], axis=1)     # sign-folded
    sin2 = np.ascontiguousarray(np.tile(sinf.T, (2, 1)))
    j = np.arange(128)[:, None, None]
    c = np.arange(128)[None, None, :]
    tri2 = np.broadcast_to(j <= c, (128, 2, 128)).astype(np.float32)

    bf = ml_dtypes.bfloat16
    Wq4 = Wq.reshape(D, H, HD)
    Wo4 = Wo.reshape(H, HD, D)
    Wk4 = Wk.reshape(D, KV, HD)
    Wv4 = Wv.reshape(D, KV, HD)

    maps = []
    for core in range(N_CORES):
        b, g2 = core // 4, core % 4
        gh = [8 * g2 + p for p in PERM]
        w6 = np.concatenate([
            Wq4[:, gh, :].reshape(D, 512),
            Wk4[:, [2 * g2, 2 * g2 + 1], :].reshape(D, 128),
            Wv4[:, [2 * g2, 2 * g2 + 1], :].reshape(D, 128),
        ], axis=1)
        maps.append({
            "xT": np.ascontiguousarray(x[b].T).astype(bf),
            "w6": np.ascontiguousarray(w6).astype(bf),
            "wo": np.ascontiguousarray(Wo4[gh].reshape(512, D)).astype(bf),
            "cos2": cos2.astype(bf), "sin2": sin2.astype(bf),
            "tri2": tri2.astype(bf),
        })
    return maps


_NC_CACHE = None


def kernel(x, Wq, Wk, Wv, Wo):
    global LAST_RESULT, _NC_CACHE
    maps = host_inputs(np.asarray(x, np.float32), np.asarray(Wq, np.float32),
                       np.asarray(Wk, np.float32), np.asarray(Wv, np.float32),
                       np.asarray(Wo, np.float32))
    if _NC_CACHE is None:
        _NC_CACHE = build_nc()
    trace = bool(os.environ.get("KERNEL_TRACE"))
    try:
        res = run_bass_kernel_spmd(_NC_CACHE, maps, list(range(N_CORES)), trace=trace)
    except (ImportError, ModuleNotFoundError):
        res = run_bass_kernel_spmd(_NC_CACHE, maps, list(range(N_CORES)), trace=False)
    LAST_RESULT = res
    out = np.empty((B, S, D), np.float32)
    for b in range(B):
        out[b] = res.results[4 * b]["y"]
        for g2 in range(1, 4):
            out[b] += res.results[4 * b + g2]["y"]
    return out


# revision 5
# speedup vs baseline: 1.4162x; 1.0023x over previous
"""GQA kernel for trn2, 8 NeuronCores — v2 (bf16, phase-interleaved).

Sharding: core c = (b, g2), b = c//4, g2 = c%4: batch b, kv heads {2g2, 2g2+1},
q heads 8g2..8g2+7 permuted [0,4,1,5,2,6,3,7] into 4 pairs t (rows 0:64 = kv0's
q head, 64:128 = kv1's).  Host sums the 4 partial y outputs per batch.

Dataflow (all matmuls bf16 in / f32 psum out):
  A: qkT[e,s] = W.T x; sb0 runs d-outer (5 qk accumulators across acc+sc
     banks, consuming each x d-tile on arrival at cold-start DMA pace, split
     across the sync and gpsimd DGE queues); later sbs run e-output-major per
     256-col half-sb on a 2-bank acc ring.  v is computed in natural [s,e]
     orientation (no PE transpose).  RoPE: ACT drains acc->qk5 (packed
     [128,5,256] bf16), 4 gpsimd shift-DMAs rotate all 5 tensors at once,
     3 DVE ops apply cos/sin (broadcast AP).
  B: per (bi,t): fused scores psum [128,2,512] (slot A/B in different banks
     -- one matmul accumulation group per 2KB PSUM bank is a hard HW rule),
     one fused 3D exp per j-tile, DVE causal mask on diagonal, pv accumulated
     per (bi,t) into pvA[65,512] (rowsumA in row 64) / pvB[128,512] (rowsumB
     in row 32, out rows 64:128 -- partition-aligned with outT, no DMA moves).
     Normalize: DVE reciprocal (bf16) -> two PE broadcast matmuls (stationary
     ones row at partition 64/32) into psum tiles borrowed from the acc/yp
     ring -> DVE copies -> DVE muls -> outT bf16.  The bc chain never touches
     a DMA queue, and is deferred past a filler unit so the pv ring frees.
  C: y[s,d] = outT.T wo per (stt,db), DVE (ACT in the tail) drain, sync DMA.

Emission interleaves phases so the in-order PE queue never starves on ACT
(exp) or normalization chains: A units and C db-groups are injected as
fillers INSIDE B's jt loops (ACT exp is ~190ns/jt slower than B's PE work):
  A0(d-outer) A1 | B0t*xA2 | B1t*xA3 | B2t*xC(stt0-7) | B3t*xC(stt8-11) |
  C(stt12-15) tail.
PSUM: acc 2 banks (A, closes after A3) -> yp 2 banks (C); sc 4; pvA 1; pvB 1.
"""

import os
import numpy as np
import ml_dtypes

import concourse.bass as bass
import concourse.bacc as bacc
import concourse.mybir as mybir
import concourse.tile as tile
from concourse.bass_utils import run_bass_kernel_spmd

F32 = mybir.dt.float32
BF16 = mybir.dt.bfloat16

B, S, D = 2, 2048, 2048
H, KV, HD = 32, 8, 64
N_CORES = 8
NDT = D // 128          # 16 d-tiles
NIT = 4                 # 512-wide i-blocks
PERM = [0, 4, 1, 5, 2, 6, 3, 7]

LAST_RESULT = None


def build_nc():
    nc = bacc.Bacc("TRN2", target_bir_lowering=False, debug=False,
                   enable_asserts=True, num_devices=N_CORES)

    xT = nc.dram_tensor("xT", [D, S], BF16, kind="ExternalInput")
    w6 = nc.dram_tensor("w6", [D, 768], BF16, kind="ExternalInput")
    wo = nc.dram_tensor("wo", [512, D], BF16, kind="ExternalInput")
    cos2 = nc.dram_tensor("cos2", [128, S], BF16, kind="ExternalInput")
    sin2 = nc.dram_tensor("sin2", [128, S], BF16, kind="ExternalInput")
    tri2 = nc.dram_tensor("tri2", [128, 2, 128], BF16, kind="ExternalInput")
    y = nc.dram_tensor("y", [S, D], F32, kind="ExternalOutput")

    with tile.TileContext(nc) as tc:
        with (
            tc.tile_pool(name="persist", bufs=1) as persist,
            tc.tile_pool(name="xpool", bufs=32) as xpool,
            tc.tile_pool(name="qk5p", bufs=2) as qk5p,
            tc.tile_pool(name="rot5p", bufs=2) as rot5p,
            tc.tile_pool(name="tp", bufs=2) as tp,
            tc.tile_pool(name="epool", bufs=4) as epool,
            tc.tile_pool(name="recp", bufs=2) as recp,
            tc.tile_pool(name="bcp", bufs=2) as bcp,
            tc.tile_pool(name="ypool", bufs=4) as ypool,
            tc.tile_pool(name="bpsum", bufs=1, space="PSUM") as bpsum,
        ):
            # ---- persistent SBUF ----
            qkT = persist.tile([128, 5, S], BF16, name="qkT")
            vA = persist.tile([128, 16, 65], BF16, name="vA")
            vB = persist.tile([128, 16, 128], BF16, name="vB")
            outT = [persist.tile([128, S], BF16, name=f"outT{t}") for t in range(4)]
            cos_sb = persist.tile([128, S], BF16, name="cos_sb")
            sin_sb = persist.tile([128, S], BF16, name="sin_sb")
            tri_sb = persist.tile([128, 2, 128], BF16, name="tri_sb")
            w6_sb = [persist.tile([128, 768], BF16, name=f"w6_{d}") for d in range(NDT)]
            wo_sb = [persist.tile([128, D], BF16, name=f"wo{f}") for f in range(4)]

            ones64 = persist.tile([128, 64], BF16, name="ones64")

            def emit_memsets():
                nc.gpsimd.memset(vB[:], 0.0)
                nc.gpsimd.memset(vA[:, :, 64:65], 1.0)
                nc.gpsimd.memset(vB[:, :, 32:33], 1.0)
                nc.gpsimd.memset(ones64[:], 1.0)

            xt = [[None] * NDT for _ in range(4)]

            def emit_dma_A(sb):
                # sb0: alternate the two DGE queues so the d-tile stream
                # arrives ~2x faster during the cold start.
                for d in range(NDT):
                    if sb == 0:
                        eng = nc.sync if d % 2 == 0 else nc.gpsimd
                        eng.dma_start(w6_sb[d][:], w6[d * 128:(d + 1) * 128, :])
                        if d == 4:
                            nc.sync.dma_start(cos_sb[:], cos2[:])
                            nc.sync.dma_start(sin_sb[:], sin2[:])
                        if d == 8:
                            nc.sync.dma_start(tri_sb[:], tri2[:])
                    t_ = xpool.tile([128, 512], BF16, name="xt", tag="xt")
                    eng = nc.gpsimd if (sb == 0 and d % 2 == 0) else nc.sync
                    eng.dma_start(t_[:], xT[d * 128:(d + 1) * 128,
                                            sb * 512:(sb + 1) * 512])
                    xt[sb][d] = t_

            def emit_A0_douter(apool):
                """Full-width d-outer qk accumulation for sb0: consume each
                arriving x d-tile across all 5 qk outputs (512 cols each) so
                PE keeps pace with the serialized cold-start DMA stream.
                Borrows the idle sc banks for three extra accumulators."""
                accK = apool.tile([128, 512], F32, name="acc", tag="acc")
                accQ0 = apool.tile([128, 512], F32, name="acc", tag="acc")
                scT1 = bpsum.tile([128, 2, 512], F32, name="sc", tag="sc", bufs=2)
                scT2 = bpsum.tile([128, 2, 512], F32, name="sc", tag="sc", bufs=2)
                qk_dst = [accK[:, :], accQ0[:, :], scT1[:, 0, :], scT1[:, 1, :],
                          scT2[:, 0, :]]
                for d in range(NDT):
                    st, sp = d == 0, d == NDT - 1
                    for o in range(5):
                        ecol = (slice(512, 640) if o == 0
                                else slice((o - 1) * 128, o * 128))
                        nc.tensor.matmul(qk_dst[o], w6_sb[d][:, ecol],
                                         xt[0][d][:, 0:512], start=st, stop=sp)
                for hsb in range(2):
                    scol = slice(hsb * 256, hsb * 256 + 256)
                    qk5 = qk5p.tile([128, 5, 256], BF16, name="qk5", tag="qk5")
                    for o in range(5):
                        nc.scalar.copy(qk5[:, o, :], qk_dst[o][:, scol])
                    rot5 = rot5p.tile([128, 5, 256], BF16, name="rot5",
                                      tag="rot5")
                    for (a, b_) in ((0, 32), (32, 0), (64, 96), (96, 64)):
                        nc.gpsimd.dma_start(rot5[b_:b_ + 32, :, :],
                                            qk5[a:a + 32, :, :])
                    t1 = tp.tile([128, 5, 256], BF16, name="t1", tag="t1")
                    nc.vector.tensor_mul(
                        t1[:], qk5[:],
                        cos_sb[:, scol].unsqueeze(1).broadcast_to((128, 5, 256)))
                    t2 = tp.tile([128, 5, 256], BF16, name="t2", tag="t2")
                    nc.vector.tensor_mul(
                        t2[:], rot5[:],
                        sin_sb[:, scol].unsqueeze(1).broadcast_to((128, 5, 256)))
                    nc.vector.tensor_add(qkT[:, :, scol], t1[:], t2[:])
                for u in range(4):
                    acc = apool.tile([128, 512], F32, name="vacc", tag="acc")
                    for d in range(NDT):
                        nc.tensor.matmul(acc[:, 0:128],
                                         xt[0][d][:, u * 128:(u + 1) * 128],
                                         w6_sb[d][:, 640:768],
                                         start=(d == 0), stop=(d == NDT - 1))
                    nc.vector.tensor_copy(vA[:, u, 0:64], acc[:, 0:64])
                    nc.vector.tensor_copy(vB[:, u, 64:128], acc[:, 64:128])

            def a_units(sb, apool, hsbs=(0, 1)):
                """Yield per-unit emit closures for phase A of s-block sb."""
                for hsb in hsbs:
                    scol = slice(sb * 512 + hsb * 256, sb * 512 + hsb * 256 + 256)
                    hcol = slice(hsb * 256, hsb * 256 + 256)
                    qk5 = [None]

                    def qk_unit(o, sb=sb, hsb=hsb, hcol=hcol, qk5=qk5):
                        if o == 0:
                            qk5[0] = qk5p.tile([128, 5, 256], BF16, name="qk5",
                                               tag="qk5")
                        ecol = (slice(512, 640) if o == 0
                                else slice((o - 1) * 128, o * 128))
                        acc = apool.tile([128, 512], F32, name="acc", tag="acc")
                        for d in range(NDT):
                            nc.tensor.matmul(acc[:, 0:256], w6_sb[d][:, ecol],
                                             xt[sb][d][:, hcol],
                                             start=(d == 0), stop=(d == NDT - 1))
                        nc.scalar.copy(qk5[0][:, o, :], acc[:, 0:256])

                    def v_unit(u, sb=sb, hsb=hsb):
                        jt = sb * 4 + hsb * 2 + u
                        sc128 = slice(hsb * 256 + u * 128, hsb * 256 + u * 128 + 128)
                        acc = apool.tile([128, 512], F32, name="vacc", tag="acc")
                        for d in range(NDT):
                            nc.tensor.matmul(acc[:, 0:128], xt[sb][d][:, sc128],
                                             w6_sb[d][:, 640:768],
                                             start=(d == 0), stop=(d == NDT - 1))
                        nc.vector.tensor_copy(vA[:, jt, 0:64], acc[:, 0:64])
                        nc.vector.tensor_copy(vB[:, jt, 64:128], acc[:, 64:128])

                    def rope_unit(scol=scol, qk5=qk5):
                        rot5 = rot5p.tile([128, 5, 256], BF16, name="rot5",
                                          tag="rot5")
                        for (a, b_) in ((0, 32), (32, 0), (64, 96), (96, 64)):
                            nc.gpsimd.dma_start(rot5[b_:b_ + 32, :, :],
                                                qk5[0][a:a + 32, :, :])
                        t1 = tp.tile([128, 5, 256], BF16, name="t1", tag="t1")
                        nc.vector.tensor_mul(
                            t1[:], qk5[0][:],
                            cos_sb[:, scol].unsqueeze(1).broadcast_to((128, 5, 256)))
                        t2 = tp.tile([128, 5, 256], BF16, name="t2", tag="t2")
                        nc.vector.tensor_mul(
                            t2[:], rot5[:],
                            sin_sb[:, scol].unsqueeze(1).broadcast_to((128, 5, 256)))
                        nc.vector.tensor_add(qkT[:, :, scol], t1[:], t2[:])

                    for o in range(5):
                        yield lambda o=o, f=qk_unit: f(o)
                    yield lambda f=rope_unit: f()
                    for u in range(2):
                        yield lambda u=u, f=v_unit: f(u)

            def emit_B_unit(bi, t, bcpool, bctag, filler=None):
                icol = slice(bi * 512, (bi + 1) * 512)
                njt = 4 * bi + 4
                fill_at = ({3, 7} if bi == 2 else
                           {2, 5, 8, 11, 14} if bi == 3 else {3, 7, 11})
                pvA = bpsum.tile([65, 512], F32, name="pvA", tag="pvA")
                pvB = bpsum.tile([128, 512], F32, name="pvB", tag="pvB")
                for jt in range(njt):
                    if filler is not None and jt in fill_at:
                        filler()
                    jcol = slice(jt * 128, (jt + 1) * 128)
                    ro = jt - 4 * bi
                    lo = 128 * max(ro, 0)
                    iband = slice(bi * 512 + lo, (bi + 1) * 512)
                    sc = bpsum.tile([128, 2, 512], F32, name="sc", tag="sc",
                                    bufs=2)
                    nc.tensor.matmul(sc[:, 0, lo:], qkT[0:64, 0, jcol],
                                     qkT[0:64, 1 + t, iband],
                                     start=True, stop=True)
                    nc.tensor.matmul(sc[:, 1, lo:], qkT[64:128, 0, jcol],
                                     qkT[64:128, 1 + t, iband],
                                     start=True, stop=True)
                    e = epool.tile([128, 2, 512], BF16, name="e", tag="e")
                    nc.scalar.activation(e[:, :, lo:], sc[:, :, lo:],
                                         mybir.ActivationFunctionType.Exp,
                                         scale=0.125)
                    if ro >= 0:
                        nc.vector.tensor_mul(e[:, :, lo:lo + 128],
                                             e[:, :, lo:lo + 128], tri_sb[:])
                    st, sp = jt == 0, jt == njt - 1
                    nc.tensor.matmul(pvA[:, lo:], vA[:, jt, :], e[:, 0, lo:],
                                     start=st, stop=sp)
                    nc.tensor.matmul(pvB[:, lo:], vB[:, jt, :], e[:, 1, lo:],
                                     start=st, stop=sp)
                # normalization: recip (DVE) now; bc matmuls + muls deferred.
                rec = recp.tile([128, 512], BF16, name="rec", tag="rec")
                with nc.allow_low_precision(reason="bf16 recip, 0.4% ok at 2e-2 tol"):
                    nc.vector.reciprocal(rec[64:65, :], pvA[64:65, :])
                    nc.vector.reciprocal(rec[32:33, :], pvB[32:33, :])

                def tail():
                    bcP1 = bcpool.tile([128, 512], F32, name="bcP1", tag=bctag)
                    nc.tensor.matmul(bcP1[0:64, :], ones64[64:65, :],
                                     rec[64:65, :], start=True, stop=True)
                    bcP2 = bcpool.tile([128, 512], F32, name="bcP2", tag=bctag)
                    nc.tensor.matmul(bcP2[64:128, :], ones64[32:33, :],
                                     rec[32:33, :], start=True, stop=True)
                    bcS = bcp.tile([128, 512], F32, name="bcS", tag="bcS")
                    nc.vector.tensor_copy(bcS[0:64, :], bcP1[0:64, :])
                    nc.vector.tensor_copy(bcS[64:128, :], bcP2[64:128, :])
                    nc.vector.tensor_mul(outT[t][0:64, icol], pvA[0:64, :],
                                         bcS[0:64, :])
                    nc.vector.tensor_mul(outT[t][64:128, icol], pvB[64:128, :],
                                         bcS[64:128, :])
                return tail

            def emit_C_group(stt, db, cpool, tail=False):
                srow = slice(stt * 128, (stt + 1) * 128)
                dcol = slice(db * 512, (db + 1) * 512)
                yp = cpool.tile([128, 512], F32, name="yp", tag="yp")
                for f in range(4):
                    nc.tensor.matmul(yp[:], outT[f][:, srow],
                                     wo_sb[f][:, dcol],
                                     start=(f == 0), stop=(f == 3))
                ys = ypool.tile([128, 512], F32, name="ys", tag="ys")
                if tail and db % 2 == 0:
                    nc.scalar.copy(ys[:], yp[:])
                else:
                    nc.vector.tensor_copy(ys[:], yp[:])
                nc.sync.dma_start(y[srow, dcol], ys[:])

            # ================= emission schedule =================
            from collections import deque

            with tc.tile_pool(name="apsum", bufs=2, space="PSUM") as apool:
                emit_memsets()
                emit_dma_A(0)
                emit_A0_douter(apool)
                emit_dma_A(1)
                for u in a_units(1, apool):
                    u()
                emit_dma_A(2)
                afill = deque(a_units(2, apool))
                fa = lambda: afill.popleft()() if afill else None
                for t in range(4):
                    nc.sync.dma_start(wo_sb[t][:], wo[t * 128:(t + 1) * 128, :])
                    tail = emit_B_unit(0, t, apool, "acc", filler=fa)
                    fa()
                    tail()
                    fa()
                emit_dma_A(3)
                afill.extend(a_units(3, apool))
                for t in range(4):
                    tail = emit_B_unit(1, t, apool, "acc", filler=fa)
                    fa()
                    tail()
                    fa()
                    fa()
                while afill:
                    fa()
            with tc.tile_pool(name="cpsum", bufs=2, space="PSUM") as cpool:
                cfill = deque((stt, db) for stt in range(8) for db in range(4))
                fc = lambda: (emit_C_group(*cfill.popleft(), cpool)
                              if cfill else None)
                for t in range(4):
                    tail = emit_B_unit(2, t, cpool, "yp", filler=fc)
                    fc()
                    fc()
                    tail()
                    fc()
                cfill.extend((stt, db) for stt in range(8, 12) for db in range(4))
                for t in range(4):
                    tail = emit_B_unit(3, t, cpool, "yp", filler=fc)
                    fc()
                    tail()
                    fc()
                for stt in range(12, 16):
                    for db in range(4):
                        emit_C_group(stt, db, cpool, tail=True)

    nc.compile()
    return nc


def host_inputs(x, Wq, Wk, Wv, Wo):
    inv = 1.0 / (10000.0 ** (np.arange(0, HD, 2, dtype=np.float64) / HD))
    freqs = np.outer(np.arange(S, dtype=np.float64), inv)          # [S, 32]
    emb = np.concatenate([freqs, freqs], axis=1)                   # [S, 64]
    cos = np.cos(emb).astype(np.float32)
    sin = np.sin(emb).astype(np.float32)
    cos2 = np.ascontiguousarray(np.tile(cos.T, (2, 1)))            # [128, S]
    sinf = np.concatenate([-sin[:, :32], sin[:, 32:]], axis=1)     # sign-folded
    sin2 = np.ascontiguousarray(np.tile(sinf.T, (2, 1)))
    j = np.arange(128)[:, None, None]
    c = np.arange(128)[None, None, :]
    tri2 = np.broadcast_to(j <= c, (128, 2, 128)).astype(np.float32)

    bf = ml_dtypes.bfloat16
    Wq4 = Wq.reshape(D, H, HD)
    Wo4 = Wo.reshape(H, HD, D)
    Wk4 = Wk.reshape(D, KV, HD)
    Wv4 = Wv.reshape(D, KV, HD)

    maps = []
    for core in range(N_CORES):
        b, g2 = core // 4, core % 4
        gh = [8 * g2 + p for p in PERM]
        w6 = np.concatenate([
            Wq4[:, gh, :].reshape(D, 512),
            Wk4[:, [2 * g2, 2 * g2 + 1], :].reshape(D, 128),
            Wv4[:, [2 * g2, 2 * g2 + 1], :].reshape(D, 128),
        ], axis=1)
        maps.append({
            "xT": np.ascontiguousarray(x[b].T).astype(bf),
            "w6": np.ascontiguousarray(w6).astype(bf),
            "wo": np.ascontiguousarray(Wo4[gh].reshape(512, D)).astype(bf),
            "cos2": cos2.astype(bf), "sin2": sin2.astype(bf),
            "tri2": tri2.astype(bf),
        })
    return maps


_NC_CACHE = None


def kernel(x, Wq, Wk, Wv, Wo):
    global LAST_RESULT, _NC_CACHE
    maps = host_inputs(np.asarray(x, np.float32), np.asarray(Wq, np.float32),
                       np.asarray(Wk, np.float32), np.asarray(Wv, np.float32),
                       np.asarray(Wo, np.float32))
    if _NC_CACHE is None:
        _NC_CACHE = build_nc()
    trace = bool(os.environ.get("KERNEL_TRACE"))
    try:
        res = run_bass_kernel_spmd(_NC_CACHE, maps, list(range(N_CORES)), trace=trace)
    except (ImportError, ModuleNotFoundError):
        res = run_bass_kernel_spmd(_NC_CACHE, maps, list(range(N_CORES)), trace=False)
    LAST_RESULT = res
    out = np.empty((B, S, D), np.float32)
    for b in range(B):
        out[b] = res.results[4 * b]["y"]
        for g2 in range(1, 4):
            out[b] += res.results[4 * b + g2]["y"]
    return out
